# revision 42
# baseline (speedup 1.0000x reference)
"""Trainium2 Bass kernel: multi-head attention (B=8, N=1024, C=768, H=12).

Sharding: pure data-parallel — batch dim (8) maps 1:1 onto the 8 NeuronCores;
weights are replicated. No collectives.

Per-core algorithm (one batch element, all compute in bf16 w/ fp32 PSUM accum):
  1. qkT = [wq*scale; wk] @ x.T            -> [1536, 1024]  (head-dim on partitions)
  2. V   = x @ wv.T                        -> [1024, 768]   (tokens on partitions)
     stored interleaved with a ones-column per head ("Vaug", [*, 65] per head)
  3. per head h: S.T tiles = kT_h.T-matmul (K=64, two heads row-packed via
     tile_position) -> exp on ScalarE -> P.T (unnormalized, bf16)
  4. AV: out.T_h(+denom) = [V_h | 1].T-matmul-accum over nk   (M=65)
     row 64 = softmax denominator; normalize rows 0..63 via
     reciprocal_approx_fast + partition-broadcast DMA + VectorE multiply
  5. y.T = wp.T-matmul(outT) + bias        -> [768, 1024] fp32 -> DRAM

Host side transposes x / weights into the layouts above (bf16) and transposes
the [768, 1024] per-core outputs back into [8, 1024, 768] fp32.
"""
import sys

sys.path.insert(0, "/opt/trn_rl_repo")

import numpy as np
import ml_dtypes

import concourse.bass as bass  # noqa: F401  (registers AP helpers)
import concourse.mybir as mybir
import concourse.tile as tile
from concourse import bacc
from concourse.bass_utils import run_bass_kernel_spmd

B, N, C, H, HD = 8, 1024, 768, 12, 64
KC = C // 128          # 6   contraction chunks of 128 over C
FT = (2 * C) // 128    # 12  q+k feature tiles of 128
NT = N // 128          # 8   token tiles of 128
NQ = N // 512          # 2   query chunks of 512
G = H // 2             # 6   head pairs
BF16 = mybir.dt.bfloat16
F32 = mybir.dt.float32
EXP = mybir.ActivationFunctionType.Exp

_CACHE = {}


def _build(dbg=False):
    nc = bacc.Bacc("TRN2", target_bir_lowering=False, debug=False,
                   enable_asserts=False, num_devices=B)
    xt_d = nc.dram_tensor("xt", [C, N], BF16, kind="ExternalInput").ap()
    wqk_d = nc.dram_tensor("wqk", [C, 2 * C], BF16, kind="ExternalInput").ap()
    wv_d = nc.dram_tensor("wv", [C, C], BF16, kind="ExternalInput").ap()
    wp_d = nc.dram_tensor("wp", [C, C], BF16, kind="ExternalInput").ap()
    bp_d = nc.dram_tensor("bp", [128, KC], F32, kind="ExternalInput").ap()
    out_d = nc.dram_tensor("out", [C, N], F32, kind="ExternalOutput").ap()
    if dbg:
        dbg_qk = nc.dram_tensor("dbg_qk", [128, FT * N], BF16, kind="ExternalOutput").ap()
        dbg_v = nc.dram_tensor("dbg_v", [128, NT * H * 65], BF16, kind="ExternalOutput").ap()
        dbg_pt = nc.dram_tensor("dbg_pt", [128, H * NT * 1024], BF16, kind="ExternalOutput").ap()
        dbg_outT = nc.dram_tensor("dbg_outT", [128, KC * N], BF16, kind="ExternalOutput").ap()
        dbg_av = nc.dram_tensor("dbg_av", [128, G * 2 * 512], F32, kind="ExternalOutput").ap()
        dbg_rb = nc.dram_tensor("dbg_rb", [128, G * 2 * 512], F32, kind="ExternalOutput").ap()

    with tile.TileContext(nc) as tc:
        with (
            tc.tile_pool(name="wpool", bufs=1) as wpool,
            tc.tile_pool(name="big", bufs=1) as big,
            tc.tile_pool(name="pt", bufs=20) as ptpool,
            tc.tile_pool(name="small", bufs=6) as small,
            tc.tile_pool(name="ye", bufs=3) as yepool,
            tc.tile_pool(name="psA", bufs=1, space="PSUM") as psA,
            tc.tile_pool(name="psS", bufs=2, space="PSUM") as psS,
            tc.tile_pool(name="psV", bufs=3, space="PSUM") as psV,
        ):
            xt = wpool.tile([128, KC * N], BF16)        # x.T  chunks
            wqk = wpool.tile([128, KC * 2 * C], BF16)
            wv = wpool.tile([128, KC * C], BF16)
            wp = wpool.tile([128, KC * C], BF16)
            bp = wpool.tile([128, KC], F32)
            qk = big.tile([128, FT * N], BF16)          # qkT feature tiles
            vsb = big.tile([128, NT * C], BF16)         # V: (nt, g) -> 128 cols
            ones = big.tile([128, 1], BF16)
            outTs = [big.tile([128, N], BF16, tag=f"outT{g}", name=f"outT{g}")
                     for g in range(G)]

            # spread input loads over independent DMA queues. wqk's DRAM
            # layout is host-interleaved by head pair ([q_g | k_g] blocks of
            # 256 cols) and loaded pair-major so pair 0's weights land first.
            for k in range(KC):
                nc.sync.dma_start(out=xt[:, k * N:(k + 1) * N],
                                  in_=xt_d[k * 128:(k + 1) * 128, :])
            for g in range(G):
                for k in range(KC):
                    nc.gpsimd.dma_start(
                        out=wqk[:, k * 2 * C + g * 256: k * 2 * C + (g + 1) * 256],
                        in_=wqk_d[k * 128:(k + 1) * 128, g * 256:(g + 1) * 256])
            for k in range(KC):
                nc.scalar.dma_start(out=wv[:, k * C:(k + 1) * C],
                                    in_=wv_d[k * 128:(k + 1) * 128, :])
                nc.sync.dma_start(out=wp[:, k * C:(k + 1) * C],
                                  in_=wp_d[k * 128:(k + 1) * 128, :])
            nc.scalar.dma_start(out=bp[:], in_=bp_d)

            nc.vector.memset(ones[:], 1.0)

            # PE warm-up: ~32 junk matmuls on a zeroed scratch tile keep the
            # TensorE HAM busy while the input DMAs land, so the first real
            # matmuls run at 2.4 GHz instead of 1.2 GHz.
            wu = big.tile([128, 512], BF16, name="wu")
            nc.vector.memset(wu[:], 0.0)
            for _ in range(4):
                wups = psA.tile([128, 512], F32, tag="psA", name="wups")
                for _ in range(8):
                    nc.tensor.matmul(wups[:], lhsT=wu[:, 0:128], rhs=wu[:, 0:512],
                                     start=True, stop=True)

            def qkT_tiles(t, pool=None):
                # wqk cols are pair-interleaved: q_g at g*256, k_g at g*256+128
                bcol = t * 256 if t < 6 else (t - 6) * 256 + 128
                for nq in range(NQ):
                    p = pool or psA
                    ps = p.tile([128, 512], F32, tag=p.name, name=f"qkp_{t}_{nq}")
                    for k in range(KC):
                        nc.tensor.matmul(
                            ps[:],
                            lhsT=wqk[:, k * 2 * C + bcol: k * 2 * C + bcol + 128],
                            rhs=xt[:, k * N + nq * 512: k * N + nq * 512 + 512],
                            start=(k == 0), stop=(k == KC - 1))
                    nc.vector.tensor_copy(
                        out=qk[:, t * N + nq * 512: t * N + nq * 512 + 512],
                        in_=ps[:])

            def v_tiles(nt, pool=None):
                # V feature order h*64+d == pair-block order -> contiguous dst
                for fc in range(2):
                    p = pool or psA
                    ps = p.tile([128, 384], F32, tag=p.name, name=f"vp_{nt}_{fc}")
                    for k in range(KC):
                        nc.tensor.matmul(
                            ps[:],
                            lhsT=xt[:, k * N + nt * 128: k * N + nt * 128 + 128],
                            rhs=wv[:, k * C + fc * 384: k * C + fc * 384 + 384],
                            start=(k == 0), stop=(k == KC - 1))
                    nc.vector.tensor_copy(
                        out=vsb[:, nt * C + fc * 384: nt * C + fc * 384 + 384],
                        in_=ps[:])

            pts = {}

            def scores_pair(g, nk):
                # both halves' tiles together, matmuls alternating array
                # halves (h1,h2,h1,h2) so the strict-FIFO PE overlaps them
                # via row-group concurrency.
                pss = [psS.tile([128, 2 * 512], F32, tag="psS",
                                name=f"psS_{2 * g + half}_{nk}")
                       for half in range(2)]
                for nq in range(NQ):
                    for half in range(2):
                        nc.tensor.matmul(
                            pss[half][:, nq * 512:(nq + 1) * 512],
                            lhsT=qk[half * 64:(half + 1) * 64,
                                    (6 + g) * N + nk * 128: (6 + g) * N + nk * 128 + 128],
                            rhs=qk[half * 64:(half + 1) * 64,
                                   g * N + nq * 512: g * N + nq * 512 + 512],
                            start=True, stop=True)
                for half in range(2):
                    h = 2 * g + half
                    pt = ptpool.tile([128, 2 * 512], BF16, tag="pt",
                                     name=f"pt_{h}_{nk}")
                    nc.scalar.activation(out=pt[:], in_=pss[half][:], func=EXP)
                    pts[(h, nk)] = pt
                    if dbg:
                        off = (h * NT + nk) * 1024
                        nc.sync.dma_start(out=dbg_pt[:, off:off + 1024], in_=pt[:])

            def normalize_nq(g, dn, avp, nq):
                # dn rows {nq*64, nq*64+32} hold the two heads' denominators.
                # Build a full [128,512] reciprocal tile (lo half via
                # partition_broadcast, hi half via GpSimd cross-partition
                # copy), then one multiply covers both heads.
                rbf = small.tile([128, 512], F32, tag="rbf", name=f"rbf_{g}_{nq}")
                for half in range(2):
                    row = nq * 64 + half * 32
                    den = small.tile([1, 512], F32, tag="den",
                                     name=f"den_{g}_{nq}_{half}")
                    nc.vector.tensor_copy(out=den[:], in_=dn[row:row + 1, :])
                    rec = small.tile([1, 512], F32, tag="rec",
                                     name=f"rec_{g}_{nq}_{half}")
                    nc.vector.reciprocal_approx_fast(out=rec[:], in_=den[:])
                    if half == 0:
                        nc.gpsimd.partition_broadcast(rbf[0:64, :], rec[:])
                    else:
                        tmp = small.tile([64, 512], F32, tag="rbt",
                                         name=f"rbt_{g}_{nq}")
                        nc.gpsimd.partition_broadcast(tmp[:], rec[:])
                        nc.gpsimd.tensor_copy(out=rbf[64:128, :], in_=tmp[:])
                nc.vector.tensor_mul(out=outTs[g][:, nq * 512:(nq + 1) * 512],
                                     in0=avp[:], in1=rbf[:])
                if dbg:
                    off = (g * 2 + nq) * 512
                    av_st = small.tile([128, 512], F32, tag="avst",
                                       name=f"avst_{g}_{nq}")
                    nc.vector.tensor_copy(out=av_st[:], in_=avp[:])
                    nc.sync.dma_start(out=dbg_av[:, off:off + 512], in_=av_st[:])
                    nc.sync.dma_start(out=dbg_rb[:, off:off + 512], in_=rbf[:])

            def emit_pair(g):
                """Col-packed AV for heads 2g/2g+1 (lo/hi array halves into
                one PSUM bank), 4-col-packed ones-matmul denominators, all
                interleaved with scores for pair g+1."""
                h1, h2 = 2 * g, 2 * g + 1
                if g + 1 < G:
                    qkT_tiles(g + 1)
                    qkT_tiles(6 + g + 1)
                dn = psV.tile([128, 512], F32, tag="psV", name=f"dn_{g}")
                avs = [psV.tile([128, 512], F32, tag="psV", name=f"av_{g}_{nq}")
                       for nq in range(NQ)]
                for nk in range(NT):
                    if g + 1 < G:
                        scores_pair(g + 1, nk)
                    blk = (nk * G + g) * 128
                    vlo = vsb[:, blk: blk + 64]
                    vhi = vsb[:, blk + 64: blk + 128]
                    for nq in range(NQ):
                        s, e = nq * 512, (nq + 1) * 512
                        nc.tensor.matmul(avs[nq][0:64, :], lhsT=vlo,
                                         rhs=pts[(h1, nk)][:, s:e],
                                         start=(nk == 0), stop=(nk == NT - 1))
                        nc.tensor.matmul(avs[nq][64:128, :], lhsT=vhi,
                                         rhs=pts[(h2, nk)][:, s:e],
                                         start=(nk == 0), stop=(nk == NT - 1))
                    for nq in range(NQ):
                        s, e = nq * 512, (nq + 1) * 512
                        for half in range(2):
                            row = nq * 64 + half * 32
                            nc.tensor.matmul(
                                dn[row:row + 1, :], lhsT=ones[:],
                                rhs=pts[(2 * g + half, nk)][:, s:e],
                                start=(nk == 0), stop=(nk == NT - 1),
                                tile_position=(0, row))
                for nq in range(NQ):
                    normalize_nq(g, dn, avs[nq], nq)

            def proj():
                # stage 3 is done: psA/psV/psS banks are all free, so rotate
                # proj accumulators across the three pools (6 groups in
                # flight instead of 2) to shorten the tail.
                for t in range(KC):
                    for nq in range(NQ):
                        idx = t * NQ + nq
                        if idx % 3 == 2:
                            ps = psS.tile([128, 1024], F32, tag="psS",
                                          name=f"psP_{t}_{nq}")[:, 0:512]
                        elif idx % 3 == 1:
                            ps = psV.tile([128, 512], F32, tag="psV",
                                          name=f"psP_{t}_{nq}")
                        else:
                            ps = psA.tile([128, 512], F32, tag="psA",
                                          name=f"psP_{t}_{nq}")
                        for k in range(KC):
                            nc.tensor.matmul(
                                ps[:],
                                lhsT=wp[:, k * C + t * 128: k * C + (t + 1) * 128],
                                rhs=outTs[k][:, nq * 512: nq * 512 + 512],
                                start=(k == 0), stop=(k == KC - 1))
                        ye = yepool.tile([128, 512], F32, tag="ye")
                        nc.scalar.add(out=ye[:], in_=ps[:], add=bp[:, t:t + 1])
                        nc.sync.dma_start(
                            out=out_d[t * 128:(t + 1) * 128, nq * 512: nq * 512 + 512],
                            in_=ye[:])

            # emission order == scheduling priority: scores(0) as early as
            # possible so ScalarE (the pace-setting engine) starts exp'ing.
            # All remaining qkT tiles + V are produced while exps(0) run, so
            # the steady-state loop is purely scores -> exp -> AV with no
            # cross-pair DVE coupling on the qk tile.
            qkT_tiles(0, psA)
            qkT_tiles(6, psV)
            # pair-0 scores interleaved with V so PE fills ScalarE's pacing
            # slack instead of serializing ahead of it; head matmul groups
            # alternate psA/psV so neither pool serializes
            for nk in range(NT):
                scores_pair(0, nk)
                v_tiles(nk, psA if nk % 2 == 0 else psV)
            for g in range(G):
                emit_pair(g)
            if dbg:
                nc.sync.dma_start(out=dbg_qk[:], in_=qk[:])
                nc.sync.dma_start(out=dbg_v[:], in_=vsb[:])
                for g in range(G):
                    nc.sync.dma_start(out=dbg_outT[:, g * N:(g + 1) * N], in_=outTs[g][:])
            proj()

    nc.compile()
    return nc


def _get_nc():
    if "nc" not in _CACHE:
        _CACHE["nc"] = _build()
    return _CACHE["nc"]


def _prep_inputs(x, w_qkv, w_proj, b_proj):
    bf16 = ml_dtypes.bfloat16
    scale = np.float32(HD) ** -0.5
    # interleave [q_g | k_g] 128-col blocks per head pair g (see qkT_tiles)
    wq = (w_qkv[:C] * scale).reshape(G, 128, C)
    wk = w_qkv[C:2 * C].reshape(G, 128, C)
    wqk = np.concatenate([wq, wk], axis=1).reshape(2 * C, C)
    wqkT = np.ascontiguousarray(wqk.T).astype(bf16)
    wvT = np.ascontiguousarray(w_qkv[2 * C:].T).astype(bf16)
    wpT = np.ascontiguousarray(w_proj.T).astype(bf16)
    bpT = np.ascontiguousarray(b_proj.astype(np.float32).reshape(KC, 128).T)
    in_maps = []
    for c in range(B):
        xT = np.ascontiguousarray(x[c].T).astype(bf16)
        in_maps.append({"xt": xT, "wqk": wqkT, "wv": wvT, "wp": wpT, "bp": bpT})
    return in_maps


def run(inputs, trace=False):
    nc = _get_nc()
    in_maps = _prep_inputs(**inputs)
    res = run_bass_kernel_spmd(nc, in_maps, core_ids=list(range(B)), trace=trace)
    out = np.stack([np.asarray(res.results[c]["out"]).T for c in range(B)], axis=0)
    return np.ascontiguousarray(out.astype(np.float32)), res


def kernel(x, w_qkv, w_proj, b_proj):
    out, _ = run(dict(x=np.asarray(x), w_qkv=np.asarray(w_qkv),
                      w_proj=np.asarray(w_proj), b_proj=np.asarray(b_proj)))
    return out


# revision 43
# speedup vs baseline: 1.3405x; 1.3405x over previous
"""Trainium2 Bass kernel: multi-head attention (B=8, N=1024, C=768, H=12).

Sharding: pure data-parallel — batch dim (8) maps 1:1 onto the 8 NeuronCores;
weights are replicated. No collectives.

Per-core algorithm (one batch element, all compute in bf16 w/ fp32 PSUM accum):
  1. qkT = [wq*scale; wk] @ x.T            -> [1536, 1024]  (head-dim on partitions)
  2. V   = x @ wv.T                        -> [1024, 768]   (tokens on partitions)
     stored interleaved with a ones-column per head ("Vaug", [*, 65] per head)
  3. per head h: S.T tiles = kT_h.T-matmul (K=64, two heads row-packed via
     tile_position) -> exp on ScalarE -> P.T (unnormalized, bf16)
  4. AV: out.T_h(+denom) = [V_h | 1].T-matmul-accum over nk   (M=65)
     row 64 = softmax denominator; normalize rows 0..63 via
     reciprocal_approx_fast + partition-broadcast DMA + VectorE multiply
  5. y.T = wp.T-matmul(outT) + bias        -> [768, 1024] fp32 -> DRAM

Host side transposes x / weights into the layouts above (bf16) and transposes
the [768, 1024] per-core outputs back into [8, 1024, 768] fp32.
"""
import sys

sys.path.insert(0, "/opt/trn_rl_repo")

import numpy as np
import ml_dtypes

import concourse.bass as bass  # noqa: F401  (registers AP helpers)
import concourse.mybir as mybir
import concourse.tile as tile
from concourse import bacc
from concourse.bass_utils import run_bass_kernel_spmd

B, N, C, H, HD = 8, 1024, 768, 12, 64
KC = C // 128          # 6   contraction chunks of 128 over C
FT = (2 * C) // 128    # 12  q+k feature tiles of 128
NT = N // 128          # 8   token tiles of 128
NQ = N // 512          # 2   query chunks of 512
G = H // 2             # 6   head pairs
BF16 = mybir.dt.bfloat16
F32 = mybir.dt.float32
EXP = mybir.ActivationFunctionType.Exp

_CACHE = {}


def _build(dbg=False):
    nc = bacc.Bacc("TRN2", target_bir_lowering=False, debug=False,
                   enable_asserts=False, num_devices=B)
    xt_d = nc.dram_tensor("xt", [C, N], BF16, kind="ExternalInput").ap()
    wqk_d = nc.dram_tensor("wqk", [C, 2 * C], BF16, kind="ExternalInput").ap()
    wv_d = nc.dram_tensor("wv", [C, C], BF16, kind="ExternalInput").ap()
    wp_d = nc.dram_tensor("wp", [C, C], BF16, kind="ExternalInput").ap()
    bp_d = nc.dram_tensor("bp", [128, KC], F32, kind="ExternalInput").ap()
    out_d = nc.dram_tensor("out", [C, N], F32, kind="ExternalOutput").ap()
    if dbg:
        dbg_qk = nc.dram_tensor("dbg_qk", [128, FT * N], BF16, kind="ExternalOutput").ap()
        dbg_v = nc.dram_tensor("dbg_v", [128, NT * H * 65], BF16, kind="ExternalOutput").ap()
        dbg_pt = nc.dram_tensor("dbg_pt", [128, H * NT * 1024], BF16, kind="ExternalOutput").ap()
        dbg_outT = nc.dram_tensor("dbg_outT", [128, KC * N], BF16, kind="ExternalOutput").ap()
        dbg_av = nc.dram_tensor("dbg_av", [65, H * 2 * 512], F32, kind="ExternalOutput").ap()
        dbg_rec = nc.dram_tensor("dbg_rec", [1, H * 2 * 512], F32, kind="ExternalOutput").ap()
        dbg_rb = nc.dram_tensor("dbg_rb", [64, H * 2 * 512], F32, kind="ExternalOutput").ap()

    with tile.TileContext(nc) as tc:
        with (
            tc.tile_pool(name="wpool", bufs=1) as wpool,
            tc.tile_pool(name="big", bufs=1) as big,
            tc.tile_pool(name="pt", bufs=20) as ptpool,
            tc.tile_pool(name="small", bufs=6) as small,
            tc.tile_pool(name="ye", bufs=3) as yepool,
            tc.tile_pool(name="psA", bufs=2, space="PSUM") as psA,
            tc.tile_pool(name="psS", bufs=2, space="PSUM") as psS,
            tc.tile_pool(name="psV", bufs=2, space="PSUM") as psV,
        ):
            xt = wpool.tile([128, KC * N], BF16)        # x.T  chunks
            wqk = wpool.tile([128, KC * 2 * C], BF16)
            wv = wpool.tile([128, KC * C], BF16)
            wp = wpool.tile([128, KC * C], BF16)
            bp = wpool.tile([128, KC], F32)
            qk = big.tile([128, FT * N], BF16)          # qkT feature tiles
            vsb = big.tile([128, NT * H * 65], BF16)    # Vaug: (nt, h) -> 65 cols
            outTs = [big.tile([128, N], BF16, tag=f"outT{g}", name=f"outT{g}")
                     for g in range(G)]

            # spread input loads over independent DMA queues. wqk's DRAM
            # layout is host-interleaved by head pair ([q_g | k_g] blocks of
            # 256 cols) and loaded pair-major so pair 0's weights land first.
            for k in range(KC):
                nc.sync.dma_start(out=xt[:, k * N:(k + 1) * N],
                                  in_=xt_d[k * 128:(k + 1) * 128, :])
            for g in range(G):
                for k in range(KC):
                    nc.gpsimd.dma_start(
                        out=wqk[:, k * 2 * C + g * 256: k * 2 * C + (g + 1) * 256],
                        in_=wqk_d[k * 128:(k + 1) * 128, g * 256:(g + 1) * 256])
            for k in range(KC):
                nc.scalar.dma_start(out=wv[:, k * C:(k + 1) * C],
                                    in_=wv_d[k * 128:(k + 1) * 128, :])
                nc.sync.dma_start(out=wp[:, k * C:(k + 1) * C],
                                  in_=wp_d[k * 128:(k + 1) * 128, :])
            nc.scalar.dma_start(out=bp[:], in_=bp_d)

            v3 = vsb[:].rearrange("p (a b) -> p a b", b=65)  # a = nt*H + h
            nc.vector.memset(v3[:, :, 64:65], 1.0)           # ones columns

            # PE warm-up: ~32 junk matmuls on a zeroed scratch tile keep the
            # TensorE HAM busy while the input DMAs land, so the first real
            # matmuls run at 2.4 GHz instead of 1.2 GHz.
            wu = big.tile([128, 512], BF16, name="wu")
            nc.vector.memset(wu[:], 0.0)
            for _ in range(4):
                wups = psA.tile([128, 512], F32, tag="psA", name="wups")
                for _ in range(8):
                    nc.tensor.matmul(wups[:], lhsT=wu[:, 0:128], rhs=wu[:, 0:512],
                                     start=True, stop=True)

            def qkT_tiles(t):
                # wqk cols are pair-interleaved: q_g at g*256, k_g at g*256+128
                bcol = t * 256 if t < 6 else (t - 6) * 256 + 128
                for nq in range(NQ):
                    ps = psA.tile([128, 512], F32, tag="psA")
                    for k in range(KC):
                        nc.tensor.matmul(
                            ps[:],
                            lhsT=wqk[:, k * 2 * C + bcol: k * 2 * C + bcol + 128],
                            rhs=xt[:, k * N + nq * 512: k * N + nq * 512 + 512],
                            start=(k == 0), stop=(k == KC - 1))
                    nc.vector.tensor_copy(
                        out=qk[:, t * N + nq * 512: t * N + nq * 512 + 512],
                        in_=ps[:])

            def v_tiles(nt):
                for fc in range(2):
                    ps = psA.tile([128, 384], F32, tag="psA")
                    for k in range(KC):
                        nc.tensor.matmul(
                            ps[:],
                            lhsT=xt[:, k * N + nt * 128: k * N + nt * 128 + 128],
                            rhs=wv[:, k * C + fc * 384: k * C + fc * 384 + 384],
                            start=(k == 0), stop=(k == KC - 1))
                    dst = v3[:, nt * H + fc * 6: nt * H + fc * 6 + 6, 0:64]
                    nc.vector.tensor_copy(out=dst,
                                          in_=ps[:].rearrange("p (a b) -> p a b", b=64))

            pts = {}

            def scores_pair(g, nk):
                # both halves' tiles together, matmuls alternating array
                # halves (h1,h2,h1,h2) so the strict-FIFO PE overlaps them
                # via row-group concurrency.
                pss = [psS.tile([128, 2 * 512], F32, tag="psS",
                                name=f"psS_{2 * g + half}_{nk}")
                       for half in range(2)]
                for nq in range(NQ):
                    for half in range(2):
                        nc.tensor.matmul(
                            pss[half][:, nq * 512:(nq + 1) * 512],
                            lhsT=qk[half * 64:(half + 1) * 64,
                                    (6 + g) * N + nk * 128: (6 + g) * N + nk * 128 + 128],
                            rhs=qk[half * 64:(half + 1) * 64,
                                   g * N + nq * 512: g * N + nq * 512 + 512],
                            start=True, stop=True)
                for half in range(2):
                    h = 2 * g + half
                    pt = ptpool.tile([128, 2 * 512], BF16, tag="pt",
                                     name=f"pt_{h}_{nk}")
                    nc.scalar.activation(out=pt[:], in_=pss[half][:], func=EXP)
                    pts[(h, nk)] = pt
                    if dbg:
                        off = (h * NT + nk) * 1024
                        nc.sync.dma_start(out=dbg_pt[:, off:off + 1024], in_=pt[:])

            def av_normalize(h, ps, nq):
                # denominator row 64 -> partition 0 (shifted 1-part copy),
                # reciprocal, GpSimd partition-broadcast, then one multiply
                # straight from PSUM into outT (shifted write for odd heads).
                g, half = divmod(h, 2)
                den = small.tile([1, 512], F32, tag="den", name=f"den_{h}_{nq}")
                nc.vector.tensor_copy(out=den[:], in_=ps[64:65, :])
                rec = small.tile([1, 512], F32, tag="rec", name=f"rec_{h}_{nq}")
                nc.vector.reciprocal_approx_fast(out=rec[:], in_=den[:])
                rb = small.tile([64, 512], F32, tag="rb", name=f"rb_{h}_{nq}")
                nc.gpsimd.partition_broadcast(rb[:], rec[:])
                dst = outTs[g][half * 64:(half + 1) * 64, nq * 512: nq * 512 + 512]
                nc.vector.tensor_mul(out=dst, in0=ps[0:64, :], in1=rb[:])
                if dbg:
                    off = (h * 2 + nq) * 512
                    av_st = small.tile([65, 512], F32, tag="avst", name=f"avst_{h}_{nq}")
                    nc.vector.tensor_copy(out=av_st[:], in_=ps[0:65, :])
                    nc.sync.dma_start(out=dbg_av[:, off:off + 512], in_=av_st[:])
                    nc.sync.dma_start(out=dbg_rec[:, off:off + 512], in_=rec[:])
                    nc.sync.dma_start(out=dbg_rb[:, off:off + 512], in_=rb[:])

            def emit_pair(g):
                """AV for heads 2g/2g+1, interleaved with scores for pair g+1
                so the PE stream never parks ScalarE's exp pipeline. Two
                scores tiles lead each half (lookahead=2) so ACT restarts
                before the qkT filler; the last pair borrows the idle psS
                banks for its accumulators so the normalize chains overlap."""
                for half in range(2):
                    h = 2 * g + half
                    if g + 1 < G and half == 0:
                        qkT_tiles(g + 1)
                        qkT_tiles(6 + g + 1)
                    if g == G - 1 and half == 1:
                        # last head: borrow the freed psS banks so all four
                        # tail accumulators are in flight at once
                        pss = [psS.tile([128, 1024], F32, tag="psS",
                                        name=f"psVS_{h}_{nq}")[:, 0:512]
                               for nq in range(NQ)]
                    else:
                        pss = [psV.tile([128, 512], F32, tag="psV",
                                        name=f"psV_{h}_{nq}")
                               for nq in range(NQ)]
                    for nk in range(NT):
                        if g + 1 < G and half == 0:
                            scores_pair(g + 1, nk)
                        for nq in range(NQ):
                            nc.tensor.matmul(
                                pss[nq][0:65, :],
                                lhsT=v3[:, nk * H + h, :],
                                rhs=pts[(h, nk)][:, nq * 512:(nq + 1) * 512],
                                start=(nk == 0), stop=(nk == NT - 1))
                    for nq in range(NQ):
                        av_normalize(h, pss[nq], nq)

            def proj():
                # stage 3 is done: psA/psV/psS banks are all free, so rotate
                # proj accumulators across the three pools (6 groups in
                # flight instead of 2) to shorten the tail.
                for t in range(KC):
                    for nq in range(NQ):
                        idx = t * NQ + nq
                        if idx % 3 == 2:
                            ps = psS.tile([128, 1024], F32, tag="psS",
                                          name=f"psP_{t}_{nq}")[:, 0:512]
                        elif idx % 3 == 1:
                            ps = psV.tile([128, 512], F32, tag="psV",
                                          name=f"psP_{t}_{nq}")
                        else:
                            ps = psA.tile([128, 512], F32, tag="psA",
                                          name=f"psP_{t}_{nq}")
                        for k in range(KC):
                            nc.tensor.matmul(
                                ps[:],
                                lhsT=wp[:, k * C + t * 128: k * C + (t + 1) * 128],
                                rhs=outTs[k][:, nq * 512: nq * 512 + 512],
                                start=(k == 0), stop=(k == KC - 1))
                        ye = yepool.tile([128, 512], F32, tag="ye")
                        nc.scalar.add(out=ye[:], in_=ps[:], add=bp[:, t:t + 1])
                        nc.sync.dma_start(
                            out=out_d[t * 128:(t + 1) * 128, nq * 512: nq * 512 + 512],
                            in_=ye[:])

            # emission order == scheduling priority: scores(0) as early as
            # possible so ScalarE (the pace-setting engine) starts exp'ing.
            # All remaining qkT tiles + V are produced while exps(0) run, so
            # the steady-state loop is purely scores -> exp -> AV with no
            # cross-pair DVE coupling on the qk tile.
            qkT_tiles(0)
            qkT_tiles(6)
            # pair-0 scores interleaved with V so PE fills ScalarE's pacing
            # slack instead of serializing ahead of it
            for nk in range(NT):
                scores_pair(0, nk)
                v_tiles(nk)
            for g in range(G):
                emit_pair(g)
            if dbg:
                nc.sync.dma_start(out=dbg_qk[:], in_=qk[:])
                nc.sync.dma_start(out=dbg_v[:], in_=vsb[:])
                for g in range(G):
                    nc.sync.dma_start(out=dbg_outT[:, g * N:(g + 1) * N], in_=outTs[g][:])
            proj()

    nc.compile()
    return nc


def _get_nc():
    if "nc" not in _CACHE:
        _CACHE["nc"] = _build()
    return _CACHE["nc"]


def _prep_inputs(x, w_qkv, w_proj, b_proj):
    bf16 = ml_dtypes.bfloat16
    scale = np.float32(HD) ** -0.5
    # interleave [q_g | k_g] 128-col blocks per head pair g (see qkT_tiles)
    wq = (w_qkv[:C] * scale).reshape(G, 128, C)
    wk = w_qkv[C:2 * C].reshape(G, 128, C)
    wqk = np.concatenate([wq, wk], axis=1).reshape(2 * C, C)
    wqkT = np.ascontiguousarray(wqk.T).astype(bf16)
    wvT = np.ascontiguousarray(w_qkv[2 * C:].T).astype(bf16)
    wpT = np.ascontiguousarray(w_proj.T).astype(bf16)
    bpT = np.ascontiguousarray(b_proj.astype(np.float32).reshape(KC, 128).T)
    in_maps = []
    for c in range(B):
        xT = np.ascontiguousarray(x[c].T).astype(bf16)
        in_maps.append({"xt": xT, "wqk": wqkT, "wv": wvT, "wp": wpT, "bp": bpT})
    return in_maps


def run(inputs, trace=False):
    nc = _get_nc()
    in_maps = _prep_inputs(**inputs)
    res = run_bass_kernel_spmd(nc, in_maps, core_ids=list(range(B)), trace=trace)
    out = np.stack([np.asarray(res.results[c]["out"]).T for c in range(B)], axis=0)
    return np.ascontiguousarray(out.astype(np.float32)), res


def kernel(x, w_qkv, w_proj, b_proj):
    out, _ = run(dict(x=np.asarray(x), w_qkv=np.asarray(w_qkv),
                      w_proj=np.asarray(w_proj), b_proj=np.asarray(b_proj)))
    return out
